# revision 1
# baseline (speedup 1.0000x reference)
"""Trainium2 Bass kernel for GraphTripletGCNLayer.

Reference computation (N=100000 nodes, R=100000 rels, T=300000 triples, H=256):
    rel = rel_states[rel_idx]
    agg = zeros; agg[obj] += node[subj] + rel; agg[subj] += node[obj] + rel
    out = node + silu(concat([node, agg]) @ W + b)

Strategy (8 cores, dst-node sharded):
  - Each core owns a contiguous slab of ~N/8 destination nodes.
  - node/rel tables are replicated (bf16) in each core's DRAM; per-message
    source rows are fetched with gpsimd.dma_gather (int16 indices -> tables
    are addressed in <=32768-row chunks; messages are host-sorted by
    (group-of-windows, chunk, window) per stream).
  - Aggregation avoids scatter entirely: for each 128-dst-row window,
    agg^T accumulates in PSUM as sum over 128-message tiles of
    msg_tile^T @ onehot(d) matmuls (onehot built on DVE via iota==d).
    This yields agg already feature-major, so no transposes are needed.
  - Projection: y^T = W^T x^T over [node^T; agg^T] in bf16, then
    silu(+bias) on ScalarE, f32 residual add, and a feature-major f32
    output which the host transposes back.
"""

import sys

sys.path.insert(0, "/opt/trn_rl_repo")

import numpy as np
import ml_dtypes

import concourse.bass as bass
import concourse.bacc as bacc
import concourse.mybir as mybir
import concourse.tile as tile
from concourse.bass_utils import run_bass_kernel_spmd

BF16 = mybir.dt.bfloat16
F32 = mybir.dt.float32
I16 = mybir.dt.int16

NCORES = 8
WIN = 128          # dst rows per window (= PSUM partition count of onehot mm)
GW = 4             # windows per group (projection granularity: 512 nodes)


def _ceil(a, b):
    return -(-a // b)


def _plan(node_states, rel_states, triples):
    """Host-side message planning. Returns cfg dict + per-core arrays."""
    N, H = node_states.shape
    R = rel_states.shape[0]
    T = triples.shape[0]
    assert H == 256, H

    OWN = _ceil(N, NCORES)           # owned dst nodes per core
    WPC = _ceil(OWN, WIN)            # real windows per core
    NG = _ceil(WPC, GW)              # groups per core
    NPAD = NG * GW * WIN             # padded node columns per core

    tr = np.asarray(triples).astype(np.int64)
    s, r, o = tr[:, 0], tr[:, 1], tr[:, 2]
    # messages: (src_node, rel, dst)
    src = np.concatenate([s, o])
    rel = np.concatenate([r, r])
    dst = np.concatenate([o, s])
    owner = dst // OWN
    dl = dst - owner * OWN
    w = dl // WIN
    d = (dl - w * WIN).astype(np.float32)

    streams = {}
    for name, gidx, tabrows in (("n", src, N), ("r", rel, R)):
        NCH = _ceil(tabrows, 32768)
        CH = _ceil(tabrows, NCH)
        chunk = gidx // CH
        lidx = (gidx - chunk * CH).astype(np.int16)
        # counts[owner, chunk, window]
        counts = np.zeros((NCORES, NCH, WPC), dtype=np.int64)
        np.add.at(counts, (owner, chunk, w), 1)
        K = _ceil(counts.max(axis=0), WIN).astype(np.int64)  # [NCH, WPC] tiles
        # layout order: for g: for c: for w in group  -> rank R[c,w]
        rank = np.zeros((NCH, WPC), dtype=np.int64)
        order = []
        for g in range(NG):
            for c in range(NCH):
                for wi in range(g * GW, min((g + 1) * GW, WPC)):
                    rank[c, wi] = len(order)
                    order.append((c, wi))
        ntile_by_rank = np.array([K[c, wi] for (c, wi) in order], dtype=np.int64)
        tile_base_by_rank = np.concatenate([[0], np.cumsum(ntile_by_rank)[:-1]])
        slot_base_by_rank = tile_base_by_rank * WIN
        T_tiles = int(ntile_by_rank.sum())
        S = T_tiles * WIN

        # per-core slot assignment
        idx_cores = np.zeros((NCORES, 128, S // 16), dtype=np.int16)
        dcol_cores = np.full((NCORES, 128, max(T_tiles, 1)), -1.0,
                             dtype=np.float32)
        mrank = rank[chunk, w]
        for core in range(NCORES):
            m = owner == core
            mr = mrank[m]
            ml_ = lidx[m]
            md = d[m]
            srt = np.argsort(mr, kind="stable")
            mr = mr[srt]
            ml_ = ml_[srt]
            md = md[srt]
            # rank-run starts
            starts = np.searchsorted(mr, np.arange(len(order)))
            pos_in_run = np.arange(mr.size) - starts[mr]
            slots = slot_base_by_rank[mr] + pos_in_run
            idx_flat = np.zeros(S, dtype=np.int16)
            d_flat = np.full(S, -1.0, dtype=np.float32)
            idx_flat[slots] = ml_
            d_flat[slots] = md
            idx_cores[core] = np.tile(idx_flat.reshape(-1, 16).T, (8, 1))
            dcol_cores[core] = (
                d_flat.reshape(T_tiles, WIN).T.astype(np.float32))

        streams[name] = dict(
            NCH=NCH, CH=CH, K=K, rank=rank,
            tile_base_by_rank=tile_base_by_rank, order=order,
            T_tiles=T_tiles, S=S, idx=idx_cores, dcol=dcol_cores,
        )

    cfg = dict(N=N, R=R, H=H, T=T, OWN=OWN, WPC=WPC, NG=NG, NPAD=NPAD,
               streams=streams)
    return cfg


def _build_program(cfg):
    N, R, H = cfg["N"], cfg["R"], cfg["H"]
    WPC, NG, NPAD = cfg["WPC"], cfg["NG"], cfg["NPAD"]
    stn, str_ = cfg["streams"]["n"], cfg["streams"]["r"]

    nc = bacc.Bacc("TRN2", target_bir_lowering=False, debug=False)

    tab_n = nc.dram_tensor("tab_n", [N, H], BF16, kind="ExternalInput")
    tab_r = nc.dram_tensor("tab_r", [R, H], BF16, kind="ExternalInput")
    idx_n = nc.dram_tensor("idx_n", [128, stn["S"] // 16], I16,
                           kind="ExternalInput")
    idx_r = nc.dram_tensor("idx_r", [128, str_["S"] // 16], I16,
                           kind="ExternalInput")
    dcol_n = nc.dram_tensor("dcol_n", [128, max(stn["T_tiles"], 1)], F32,
                            kind="ExternalInput")
    dcol_r = nc.dram_tensor("dcol_r", [128, max(str_["T_tiles"], 1)], F32,
                            kind="ExternalInput")
    ndT16 = nc.dram_tensor("ndT16", [2, 128, NPAD], BF16, kind="ExternalInput")
    ndT32 = nc.dram_tensor("ndT32", [2, 128, NPAD], F32, kind="ExternalInput")
    w_blk = nc.dram_tensor("w_blk", [128, 8 * 128], BF16, kind="ExternalInput")
    b_blk = nc.dram_tensor("b_blk", [128, 2], F32, kind="ExternalInput")
    iota_d = nc.dram_tensor("iota_d", [128, 128], BF16, kind="ExternalInput")
    yT = nc.dram_tensor("yT", [2, 128, NPAD], F32, kind="ExternalOutput")

    dram = dict(n=(tab_n, idx_n, dcol_n), r=(tab_r, idx_r, dcol_r))
    NWIN_GRP = GW * WIN  # node columns per group

    with tile.TileContext(nc) as tc:
        with (
            tc.tile_pool(name="const", bufs=1) as cpool,
            tc.tile_pool(name="meta", bufs=1) as mpool,
            tc.tile_pool(name="gath", bufs=2) as gpool,
            tc.tile_pool(name="oh", bufs=6) as ohpool,
            tc.tile_pool(name="aggT", bufs=2) as apool,
            tc.tile_pool(name="ndt", bufs=2) as npool,
            tc.tile_pool(name="eout", bufs=2) as epool,
            tc.tile_pool(name="pswin", bufs=3, space="PSUM") as pswin,
            tc.tile_pool(name="psy", bufs=2, space="PSUM") as psy,
        ):
            iota_sb = cpool.tile([128, 128], BF16)
            nc.sync.dma_start(iota_sb[:], iota_d[:])
            w_sb = cpool.tile([128, 8 * 128], BF16)
            nc.sync.dma_start(w_sb[:], w_blk[:])
            b_sb = cpool.tile([128, 2], F32)
            nc.sync.dma_start(b_sb[:], b_blk[:])

            meta = {}
            for sname, st in (("n", stn), ("r", str_)):
                _, idx_t, dcol_t = dram[sname]
                idx_sb = mpool.tile([128, st["S"] // 16], I16, tag=f"idx{sname}",
                                    name=f"idx{sname}")
                nc.sync.dma_start(idx_sb[:], idx_t[:])
                dcol_sb = mpool.tile([128, max(st["T_tiles"], 1)], F32,
                                     tag=f"dc{sname}", name=f"dc{sname}")
                nc.sync.dma_start(dcol_sb[:], dcol_t[:])
                meta[sname] = (idx_sb, dcol_sb)

            for g in range(NG):
                wins = list(range(g * GW, min((g + 1) * GW, WPC)))
                # ---- gathers for this group ----
                gt = {}
                for sname, st in (("n", stn), ("r", str_)):
                    tab_t, _, _ = dram[sname]
                    idx_sb, _ = meta[sname]
                    K = st["K"]
                    Tg = int(K[:, wins].sum())
                    if Tg == 0:
                        gt[sname] = None
                        continue
                    gtile = gpool.tile([128, Tg, H], BF16, tag=f"g{sname}",
                                       name=f"g{sname}")
                    off = 0
                    MAXK = 4  # <=512 idxs/call: SWDGE desc ring limit
                    for c in range(st["NCH"]):
                        Kc = int(K[c, wins].sum())
                        if Kc == 0:
                            continue
                        r0 = st["rank"][c, wins[0]]
                        slot0 = int(st["tile_base_by_rank"][r0]) * WIN
                        rows0 = c * st["CH"]
                        rows1 = min(st["CH"] * (c + 1),
                                    N if sname == "n" else R)
                        done = 0
                        while done < Kc:
                            kk = min(MAXK, Kc - done)
                            nidx = kk * WIN
                            s0 = slot0 + done * WIN
                            nc.gpsimd.dma_gather(
                                gtile[:, off + done:off + done + kk, :],
                                tab_t[rows0:rows1, :],
                                idx_sb[:, s0 // 16:(s0 + nidx) // 16],
                                nidx, nidx, H,
                            )
                            done += kk
                        off += Kc
                    gt[sname] = gtile

                # ---- per-window onehot matmuls -> aggT ----
                aggT = [apool.tile([128, NWIN_GRP], BF16, tag=f"aggT{m}",
                                   name=f"aggT{m}") for m in range(2)]
                for wi, wv in enumerate(wins):
                    # collect tiles for this window: (stream, group-local tile,
                    # global tile index)
                    tl = []
                    for sname, st in (("n", stn), ("r", str_)):
                        K = st["K"]
                        off = 0
                        for c in range(st["NCH"]):
                            for wj in wins:
                                k = int(K[c, wj])
                                if wj == wv:
                                    tb = int(st["tile_base_by_rank"][
                                        st["rank"][c, wj]])
                                    for t in range(k):
                                        tl.append((sname, off + t, tb + t))
                                off += k
                    if not tl:
                        continue
                    pw = [pswin.tile([128, WIN], F32, tag=f"pw{m}",
                                     name=f"pw{m}") for m in range(2)]
                    for i, (sname, tloc, tglob) in enumerate(tl):
                        _, dcol_sb = meta[sname]
                        oh = ohpool.tile([128, WIN], BF16, tag="oh")
                        nc.vector.tensor_scalar(
                            oh[:], iota_sb[:], dcol_sb[:, tglob:tglob + 1],
                            None, mybir.AluOpType.is_equal)
                        for m in range(2):
                            nc.tensor.matmul(
                                pw[m][:],
                                lhsT=gt[sname][:, tloc, m * 128:(m + 1) * 128],
                                rhs=oh[:],
                                start=(i == 0), stop=(i == len(tl) - 1))
                    for m in range(2):
                        nc.vector.tensor_copy(
                            aggT[m][:, wi * WIN:(wi + 1) * WIN], pw[m][:])

                # ---- projection + epilogue ----
                col0 = g * NWIN_GRP
                nt16 = []
                nt32 = []
                for m in range(2):
                    t16 = npool.tile([128, NWIN_GRP], BF16, tag=f"nt16_{m}",
                                     name=f"nt16_{m}")
                    nc.sync.dma_start(t16[:], ndT16[m, :, col0:col0 + NWIN_GRP])
                    nt16.append(t16)
                    t32 = npool.tile([128, NWIN_GRP], F32, tag=f"nt32_{m}",
                                     name=f"nt32_{m}")
                    nc.sync.dma_start(t32[:], ndT32[m, :, col0:col0 + NWIN_GRP])
                    nt32.append(t32)
                for m in range(2):
                    py = psy.tile([128, NWIN_GRP], F32)
                    for k in range(4):
                        rhs = nt16[k] if k < 2 else aggT[k - 2]
                        kb = k * 2 + m
                        nc.tensor.matmul(
                            py[:], lhsT=w_sb[:, kb * 128:(kb + 1) * 128],
                            rhs=rhs[:], start=(k == 0), stop=(k == 3))
                    eo = epool.tile([128, NWIN_GRP], F32, tag=f"eo{m}", name=f"eo{m}")
                    nc.scalar.activation(
                        eo[:], py[:], mybir.ActivationFunctionType.Silu,
                        bias=b_sb[:, m:m + 1])
                    nc.vector.tensor_add(eo[:], eo[:], nt32[m][:])
                    nc.sync.dma_start(yT[m, :, col0:col0 + NWIN_GRP], eo[:])

    nc.finalize()
    return nc


def _host_arrays(cfg, node_states, rel_states, W, b):
    N, H, OWN, NPAD = cfg["N"], cfg["H"], cfg["OWN"], cfg["NPAD"]
    node_states = np.asarray(node_states, dtype=np.float32)
    rel_states = np.asarray(rel_states, dtype=np.float32)
    W = np.asarray(W, dtype=np.float32)
    b = np.asarray(b, dtype=np.float32)

    tab_n = node_states.astype(ml_dtypes.bfloat16)
    tab_r = rel_states.astype(ml_dtypes.bfloat16)
    # W blocks: w_blk[p, (k*2+m)*128 + j] = W[k*128+p, m*128+j]
    w_blk = np.zeros((128, 8 * 128), dtype=ml_dtypes.bfloat16)
    for k in range(4):
        for m in range(2):
            kb = k * 2 + m
            w_blk[:, kb * 128:(kb + 1) * 128] = (
                W[k * 128:(k + 1) * 128, m * 128:(m + 1) * 128])
    b_blk = b.reshape(2, 128).T.astype(np.float32).copy()  # [128, 2]
    iota = np.tile(np.arange(128, dtype=np.float32)[None, :], (128, 1)
                   ).astype(ml_dtypes.bfloat16)

    in_maps = []
    for core in range(NCORES):
        lo = core * OWN
        hi = min(N, lo + OWN)
        slab = np.zeros((NPAD, H), dtype=np.float32)
        slab[: hi - lo] = node_states[lo:hi]
        sT = np.ascontiguousarray(slab.T)  # [H, NPAD]
        nd32 = sT.reshape(2, 128, NPAD)
        nd16 = nd32.astype(ml_dtypes.bfloat16)
        im = {
            "tab_n": tab_n, "tab_r": tab_r,
            "idx_n": cfg["streams"]["n"]["idx"][core],
            "idx_r": cfg["streams"]["r"]["idx"][core],
            "dcol_n": cfg["streams"]["n"]["dcol"][core],
            "dcol_r": cfg["streams"]["r"]["dcol"][core],
            "ndT16": nd16, "ndT32": nd32,
            "w_blk": w_blk, "b_blk": b_blk, "iota_d": iota,
        }
        in_maps.append(im)
    return in_maps


def kernel(node_states, rel_states, triples, W, b, _trace=False):
    cfg = _plan(node_states, rel_states, triples)
    nc = _build_program(cfg)
    in_maps = _host_arrays(cfg, node_states, rel_states, W, b)
    res = run_bass_kernel_spmd(nc, in_maps, core_ids=list(range(NCORES)),
                               trace=_trace)
    N, H, OWN, NPAD = cfg["N"], cfg["H"], cfg["OWN"], cfg["NPAD"]
    out = np.zeros((N, H), dtype=np.float32)
    for core in range(NCORES):
        yT = res.results[core]["yT"]  # [2, 128, NPAD]
        y = yT.reshape(H, NPAD).T    # [NPAD, H]
        lo = core * OWN
        hi = min(N, lo + OWN)
        out[lo:hi] = y[: hi - lo]
    if _trace:
        kernel.last_results = res
    return out



# revision 13
# speedup vs baseline: 29.9100x; 29.9100x over previous
"""Trainium2 Bass kernel for GraphTripletGCNLayer.

Reference computation (N=100000 nodes, R=100000 rels, T=300000 triples, H=256):
    rel = rel_states[rel_idx]
    agg = zeros; agg[obj] += node[subj] + rel; agg[subj] += node[obj] + rel
    out = node + silu(concat([node, agg]) @ W + b)

Strategy (8 cores, dst-node sharded). Since
    scatter(obj, node[subj]+rel) = scatter(obj, node[subj]) + scatter(obj, rel),
message formation reduces to two independent gather streams ("n": node rows,
"r": rel rows), each scattered to destination windows.

v2 design (vs v1 baseline):
  - Gather calls are per (stream, chunk, window-group) cell: ~1.5K indices per
    dma_gather, round-robined over 4 SWDGE queues (desc-gen parallelizes
    across Q7 core pairs; measured 2.3x) with an enlarged descriptor ring.
  - Slots within a cell are packed (window-sorted, tiles cut every 128 slots,
    crossing window boundaries); per-cell padding is trailing -1 indices which
    the Q7 ucode drops at runtime, so padding costs no DMA.
  - One one-hot per gather tile (not per tile-window pair), built over the
    tile's spanned windows with group-relative fp16 dst columns.
  - PSUM->SBUF agg copies and silu run on the (otherwise idle) ACT engine.
  - node slab kept in bf16 only (residual in bf16), output written bf16.
"""

import os
import sys

sys.path.insert(0, "/opt/trn_rl_repo")

import numpy as np
import ml_dtypes

import concourse.bass as bass
import concourse.bacc as bacc
import concourse.mybir as mybir
import concourse.tile as tile
from concourse.bass_utils import run_bass_kernel_spmd

BF16 = mybir.dt.bfloat16
F16 = mybir.dt.float16
F32 = mybir.dt.float32
I16 = mybir.dt.int16

NCORES = 8
WIN = 128          # dst rows per window
GW = 6             # windows per group (group = 768 dst nodes)
NQ = int(os.environ.get("KNQ", "4"))        # SWDGE queues
SCRATCH = int(os.environ.get("KSCRATCH", "65536"))   # desc ring carveout
GBUFS = 8          # gather cell buffers in rotation
MAXROWS = 32768    # int16 index addressing limit per gather chunk
MAXCALL = int(os.environ.get("KMAXCALL", "1024"))    # idx per gather call
NEGPAD = os.environ.get("KNEGPAD", "1") == "1"       # -1 trailing padding
ACTCOPY = os.environ.get("KACTCOPY", "1") == "1"     # agg copies on ACT


def _ceil(a, b):
    return -(-a // b)


def _plan_stream(src, dst, tabrows, OWN, WPC, NG):
    """Plan one gather stream. Returns layout + per-core arrays."""
    NCH = _ceil(tabrows, MAXROWS)
    CH = _ceil(tabrows, NCH)
    owner = dst // OWN
    dloc = dst - owner * OWN
    w = dloc // WIN
    g = w // GW
    wloc = w - g * GW
    dgrp = (dloc - g * GW * WIN).astype(np.float32)   # 0..GW*128-1
    chunk = src // CH
    lidx = (src - chunk * CH).astype(np.int16)
    NCELL = NG * NCH
    cell = g * NCH + chunk

    counts = np.zeros((NCORES, NCELL), dtype=np.int64)
    np.add.at(counts, (owner, cell), 1)
    Kc = _ceil(counts.max(axis=0), WIN)               # tiles per cell [NCELL]
    Kc = np.maximum(Kc, 1)
    tile_base = np.concatenate([[0], np.cumsum(Kc)[:-1]])
    T_tiles = int(Kc.sum())
    S = T_tiles * WIN

    idx_cores = np.full((NCORES, 128, S // 16), -1, dtype=np.int16)
    dcol_cores = np.full((NCORES, 128, T_tiles), -1.0, dtype=np.float32)
    span_lo = np.full(T_tiles, GW, dtype=np.int64)
    span_hi = np.full(T_tiles, -1, dtype=np.int64)

    for core in range(NCORES):
        m = owner == core
        cell_m = cell[m]
        srt = np.lexsort((wloc[m], cell_m))
        cell_s = cell_m[srt]
        lidx_s = lidx[m][srt]
        dgrp_s = dgrp[m][srt]
        wloc_s = wloc[m][srt]
        starts = np.searchsorted(cell_s, np.arange(NCELL))
        pos = np.arange(cell_s.size) - starts[cell_s]
        slots = tile_base[cell_s] * WIN + pos
        idx_flat = np.full(S, -1, dtype=np.int16)
        idx_flat[slots] = lidx_s
        dgrp_flat = np.full(S, -1.0, dtype=np.float32)
        dgrp_flat[slots] = dgrp_s
        tile_of_slot = slots // WIN
        np.minimum.at(span_lo, tile_of_slot, wloc_s)
        np.maximum.at(span_hi, tile_of_slot, wloc_s)
        idx_cores[core] = np.tile(idx_flat.reshape(-1, 16).T, (8, 1))
        dcol_cores[core] = dgrp_flat.reshape(T_tiles, WIN).T

    # tiles nobody touches (possible when Kc forced to >=1): pin span to w0
    empty = span_hi < 0
    span_lo[empty] = 0
    span_hi[empty] = 0
    if not NEGPAD:
        idx_cores[idx_cores < 0] = 0

    return dict(NCH=NCH, CH=CH, Kc=Kc, tile_base=tile_base, T_tiles=T_tiles,
                S=S, idx=idx_cores, dcol=dcol_cores,
                span_lo=span_lo, span_hi=span_hi)


def _plan(node_states, rel_states, triples):
    N, H = node_states.shape
    R = rel_states.shape[0]
    assert H == 256, H
    OWN = _ceil(N, NCORES)
    WPC = _ceil(OWN, WIN)
    NG = _ceil(WPC, GW)
    NPAD = NG * GW * WIN

    tr = np.asarray(triples).astype(np.int64)
    s, r, o = tr[:, 0], tr[:, 1], tr[:, 2]
    streams = {
        "n": _plan_stream(np.concatenate([s, o]), np.concatenate([o, s]),
                          N, OWN, WPC, NG),
        "r": _plan_stream(np.concatenate([r, r]), np.concatenate([o, s]),
                          R, OWN, WPC, NG),
    }
    return dict(N=N, R=R, H=H, OWN=OWN, WPC=WPC, NG=NG, NPAD=NPAD,
                streams=streams)


def _build_program(cfg):
    N, R, H = cfg["N"], cfg["R"], cfg["H"]
    WPC, NG, NPAD = cfg["WPC"], cfg["NG"], cfg["NPAD"]
    stn, str_ = cfg["streams"]["n"], cfg["streams"]["r"]
    NWG = GW * WIN               # dst columns per group
    CB = NWG // 2                # projection column block (448)

    nc = bacc.Bacc("TRN2", target_bir_lowering=False, debug=False,
                   num_swdge_queues=NQ, dynamic_dma_scratch_size=SCRATCH)

    tab_n = nc.dram_tensor("tab_n", [N, H], BF16, kind="ExternalInput")
    tab_r = nc.dram_tensor("tab_r", [R, H], BF16, kind="ExternalInput")
    idx_n = nc.dram_tensor("idx_n", [128, stn["S"] // 16], I16,
                           kind="ExternalInput")
    idx_r = nc.dram_tensor("idx_r", [128, str_["S"] // 16], I16,
                           kind="ExternalInput")
    dcol_n = nc.dram_tensor("dcol_n", [128, stn["T_tiles"]], F32,
                            kind="ExternalInput")
    dcol_r = nc.dram_tensor("dcol_r", [128, str_["T_tiles"]], F32,
                            kind="ExternalInput")
    ndT16 = nc.dram_tensor("ndT16", [2, 128, NPAD], BF16, kind="ExternalInput")
    w_blk = nc.dram_tensor("w_blk", [128, 8 * 128], BF16, kind="ExternalInput")
    b_blk = nc.dram_tensor("b_blk", [128, 2], F32, kind="ExternalInput")
    iota_d = nc.dram_tensor("iota_d", [128, NWG], F32, kind="ExternalInput")
    yT = nc.dram_tensor("yT", [2, 128, NPAD], BF16, kind="ExternalOutput")

    dram = dict(n=(tab_n, idx_n, dcol_n, N), r=(tab_r, idx_r, dcol_r, R))
    KMAX = max(int(stn["Kc"].max()), int(str_["Kc"].max()))
    KCALL = MAXCALL // WIN       # max tiles per gather call

    with tile.TileContext(nc) as tc:
        with (
            tc.tile_pool(name="const", bufs=1) as cpool,
            tc.tile_pool(name="meta", bufs=1) as mpool,
            tc.tile_pool(name="gath", bufs=GBUFS) as gpool,
            tc.tile_pool(name="oh", bufs=6) as ohpool,
            tc.tile_pool(name="aggT", bufs=2) as apool,
            tc.tile_pool(name="ndt", bufs=2) as npool,
            tc.tile_pool(name="eout", bufs=2) as epool,
            tc.tile_pool(name="pswin", bufs=GW, space="PSUM") as pswin,
            tc.tile_pool(name="psy", bufs=2, space="PSUM") as psy,
        ):
            iota_sb = cpool.tile([128, NWG], F32)
            nc.sync.dma_start(iota_sb[:], iota_d[:])
            w_sb = cpool.tile([128, 8 * 128], BF16)
            nc.sync.dma_start(w_sb[:], w_blk[:])
            b_sb = cpool.tile([128, 2], F32)
            nc.sync.dma_start(b_sb[:], b_blk[:])

            meta = {}
            for sname, st in (("n", stn), ("r", str_)):
                _, idx_t, dcol_t, _ = dram[sname]
                idx_sb = mpool.tile([128, st["S"] // 16], I16,
                                    tag=f"idx{sname}", name=f"idx{sname}")
                nc.sync.dma_start(idx_sb[:], idx_t[:])
                dcol_sb = mpool.tile([128, st["T_tiles"]], F32,
                                     tag=f"dc{sname}", name=f"dc{sname}")
                nc.sync.dma_start(dcol_sb[:], dcol_t[:])
                meta[sname] = (idx_sb, dcol_sb)

            qctr = 0
            cellctr = 0
            for g in range(NG):
                gww = min(GW, WPC - g * GW)   # real windows in this group
                # ---- per-cell gathers for this group ----
                cells = []   # (sname, st, gtile, K, tbase)
                for c in range(max(stn["NCH"], str_["NCH"])):
                    for sname, st in (("n", stn), ("r", str_)):
                        if c >= st["NCH"]:
                            continue
                        tab_t, _, _, tabrows = dram[sname]
                        idx_sb, _ = meta[sname]
                        ci = g * st["NCH"] + c
                        K = int(st["Kc"][ci])
                        tbase = int(st["tile_base"][ci])
                        gt = gpool.tile([128, KMAX, H], BF16, tag="g", name="g")
                        if cellctr < GBUFS:
                            # first use of this buffer: clear so padded slots
                            # can never inject NaN into the one-hot matmul
                            nc.vector.memset(gt[:], 0.0)
                        rows0 = c * st["CH"]
                        rows1 = min(st["CH"] * (c + 1), tabrows)
                        nsplit = _ceil(K, KCALL)
                        done = 0
                        for sp in range(nsplit):
                            kk = _ceil(K - done, nsplit - sp)
                            s0 = (tbase + done) * WIN
                            nidx = kk * WIN
                            nc.gpsimd.dma_gather(
                                gt[:, done:done + kk, :],
                                tab_t[rows0:rows1, :],
                                idx_sb[:, s0 // 16:(s0 + nidx) // 16],
                                nidx, nidx, H,
                                queue_num=qctr % NQ,
                            )
                            qctr += 1
                            done += kk
                        cellctr += 1
                        cells.append((sname, st, gt, K, tbase, ci))

                # ---- scatter: per-tile one-hot + matmuls into window psums ---
                pw = [pswin.tile([128, 2 * WIN], F32, tag="pw", name="pw")
                      for w in range(gww)]
                # count (tile, window) pairs per window for start/stop flags
                total = [0] * gww
                for sname, st, gt, K, tbase, ci in cells:
                    for t in range(K):
                        lo = int(st["span_lo"][tbase + t])
                        hi = int(st["span_hi"][tbase + t])
                        for w in range(lo, hi + 1):
                            total[w] += 1
                emitted = [0] * gww
                for sname, st, gt, K, tbase, ci in cells:
                    _, dcol_sb = meta[sname]
                    for t in range(K):
                        tglob = tbase + t
                        lo = int(st["span_lo"][tglob])
                        hi = int(st["span_hi"][tglob])
                        span = hi - lo + 1
                        oh = ohpool.tile([128, span * WIN], BF16, tag="oh",
                                         name="oh")
                        nc.vector.tensor_scalar(
                            oh[:], iota_sb[:, lo * WIN:(hi + 1) * WIN],
                            dcol_sb[:, tglob:tglob + 1],
                            None, mybir.AluOpType.is_equal)
                        for m in range(2):
                            for w in range(lo, hi + 1):
                                # one accumulation group per psum bank: start
                                # pending-zeroes the whole bank, so only the
                                # very first matmul starts and the very last
                                # stops (m=1 of the final tile)
                                nc.tensor.matmul(
                                    pw[w][:, m * WIN:(m + 1) * WIN],
                                    lhsT=gt[:, t, m * 128:(m + 1) * 128],
                                    rhs=oh[:, (w - lo) * WIN:(w - lo + 1) * WIN],
                                    start=(emitted[w] == 0 and m == 0),
                                    stop=(emitted[w] == total[w] - 1 and m == 1))
                        for w in range(lo, hi + 1):
                            emitted[w] += 1

                # ---- agg^T windows -> SBUF (ACT engine) ----
                aggT = []
                for m in range(2):
                    at = apool.tile([128, NWG], BF16, tag=f"aggT{m}",
                                    name=f"aggT{m}")
                    aggT.append(at)
                for w in range(gww):
                    for m in range(2):
                        if ACTCOPY:
                            nc.scalar.activation(
                                aggT[m][:, w * WIN:(w + 1) * WIN],
                                pw[w][:, m * WIN:(m + 1) * WIN],
                                mybir.ActivationFunctionType.Copy)
                        else:
                            nc.vector.tensor_copy(
                                aggT[m][:, w * WIN:(w + 1) * WIN],
                                pw[w][:, m * WIN:(m + 1) * WIN])

                # ---- projection + epilogue ----
                col0 = g * NWG
                ncols = gww * WIN
                nt16 = []
                for m in range(2):
                    t16 = npool.tile([128, NWG], BF16, tag=f"nt16_{m}",
                                     name=f"nt16_{m}")
                    nc.sync.dma_start(t16[:, 0:ncols],
                                      ndT16[m, :, col0:col0 + ncols])
                    nt16.append(t16)
                eo = []
                for m in range(2):
                    e = epool.tile([128, NWG], BF16, tag=f"eo{m}", name=f"eo{m}")
                    eo.append(e)
                nblk = _ceil(ncols, CB)
                for m in range(2):
                    for blk in range(nblk):
                        cb0 = blk * CB
                        cbw = min(CB, ncols - cb0)
                        py = psy.tile([128, CB], F32, tag="py", name="py")
                        for k in range(4):
                            rhs = nt16[k] if k < 2 else aggT[k - 2]
                            kb = k * 2 + m
                            nc.tensor.matmul(
                                py[:, 0:cbw],
                                lhsT=w_sb[:, kb * 128:(kb + 1) * 128],
                                rhs=rhs[:, cb0:cb0 + cbw],
                                start=(k == 0), stop=(k == 3))
                        nc.scalar.activation(
                            eo[m][:, cb0:cb0 + cbw], py[:, 0:cbw],
                            mybir.ActivationFunctionType.Silu,
                            bias=b_sb[:, m:m + 1])
                        nc.vector.tensor_add(
                            eo[m][:, cb0:cb0 + cbw], eo[m][:, cb0:cb0 + cbw],
                            nt16[m][:, cb0:cb0 + cbw])
                    nc.sync.dma_start(yT[m, :, col0:col0 + ncols],
                                      eo[m][:, 0:ncols])

    nc.finalize()
    return nc


def _host_arrays(cfg, node_states, rel_states, W, b):
    N, H, OWN, NPAD = cfg["N"], cfg["H"], cfg["OWN"], cfg["NPAD"]
    node_states = np.asarray(node_states, dtype=np.float32)
    rel_states = np.asarray(rel_states, dtype=np.float32)
    W = np.asarray(W, dtype=np.float32)
    b = np.asarray(b, dtype=np.float32)

    tab_n = node_states.astype(ml_dtypes.bfloat16)
    tab_r = rel_states.astype(ml_dtypes.bfloat16)
    w_blk = np.zeros((128, 8 * 128), dtype=ml_dtypes.bfloat16)
    for k in range(4):
        for m in range(2):
            kb = k * 2 + m
            w_blk[:, kb * 128:(kb + 1) * 128] = (
                W[k * 128:(k + 1) * 128, m * 128:(m + 1) * 128])
    b_blk = b.reshape(2, 128).T.astype(np.float32).copy()
    NWG = GW * WIN
    iota = np.tile(np.arange(NWG, dtype=np.float32)[None, :], (128, 1))

    in_maps = []
    for core in range(NCORES):
        lo = core * OWN
        hi = min(N, lo + OWN)
        slab = np.zeros((NPAD, H), dtype=np.float32)
        slab[: hi - lo] = node_states[lo:hi]
        sT = np.ascontiguousarray(slab.T)
        nd16 = sT.reshape(2, 128, NPAD).astype(ml_dtypes.bfloat16)
        im = {
            "tab_n": tab_n, "tab_r": tab_r,
            "idx_n": cfg["streams"]["n"]["idx"][core],
            "idx_r": cfg["streams"]["r"]["idx"][core],
            "dcol_n": cfg["streams"]["n"]["dcol"][core],
            "dcol_r": cfg["streams"]["r"]["dcol"][core],
            "ndT16": nd16,
            "w_blk": w_blk, "b_blk": b_blk, "iota_d": iota,
        }
        in_maps.append(im)
    return in_maps


def kernel(node_states, rel_states, triples, W, b, _trace=False):
    cfg = _plan(node_states, rel_states, triples)
    nc = _build_program(cfg)
    in_maps = _host_arrays(cfg, node_states, rel_states, W, b)
    res = run_bass_kernel_spmd(nc, in_maps, core_ids=list(range(NCORES)),
                               trace=_trace)
    N, H, OWN, NPAD = cfg["N"], cfg["H"], cfg["OWN"], cfg["NPAD"]
    out = np.zeros((N, H), dtype=np.float32)
    for core in range(NCORES):
        yTv = res.results[core]["yT"]      # [2, 128, NPAD] bf16
        y = yTv.astype(np.float32).reshape(H, NPAD).T
        lo = core * OWN
        hi = min(N, lo + OWN)
        out[lo:hi] = y[: hi - lo]
    if _trace:
        kernel.last_results = res
    return out
